# revision 1
# baseline (speedup 1.0000x reference)
"""Conv2d 3x3 (stride 1, pad 1) NCHW kernel for 8 Trainium2 NeuronCores.

Problem: x (32,128,56,56) f32, weight (256,128,3,3), bias (256,)
         -> out (32,256,56,56), same-padding conv + bias.

Strategy:
  - Data parallel: 4 images per core across 8 cores (batch shard).
  - Host pre-pads x to 58x58 and transposes weight to [Cin, kh*kw, Cout]
    so every DMA is large and contiguous.
  - Implicit GEMM: input channels (128) live on the SBUF partition dim.
    For each (kh, kw) of the 3x3 window the conv is a [128x128] weight
    matmul against a shifted spatial window of the padded image; the 9
    taps accumulate in PSUM.  Output tiling: 128 output channels x
    (8 rows x 56 cols) = free dim 448 per matmul (<=512 fp32 PSUM bank).
  - float32r matmuls: full PE rate at free dim >= 256 (1 cycle/row, vs 4
    for fp32), ~1.5e-4 rel err; accumulation is fp32 in PSUM.
  - Bias is fused into the PSUM->SBUF eviction (ACT/DVE alternating).
  - Startup: Tile deps are sub-tile-range aware, so the first matmul
    gates only on taps 0-3 of the oc=0 weights (ACT HWDGE ring) and
    input rows 0-9 of image 0 (SP ring), transferring in parallel.
    Stores own the SP ring, input loads the ACT ring.  The final
    group's eviction/store is split across ACT+DVE and both rings to
    shorten the kernel tail.

Measured (repeat-slope method, see bench.py): ~93 us/body steady state,
at the f32r PE roofline (504 matmuls x 448 cols / 2.4 GHz = 94.1 us);
cost-model single-exec estimate ~107 us including startup + drain tail.
"""

import os
import numpy as np

N_CORES = 8
N, C, H, W = 32, 128, 56, 56
O = 256
KH = KW = 3
PAD = 1
HP, WP = H + 2 * PAD, W + 2 * PAD  # 58, 58
NPC = N // N_CORES  # images per core = 4
RPC = 8  # output rows per chunk
N_CHUNKS = H // RPC  # 7
OC_TILES = O // 128  # 2

_CACHE = {}
LAST_RESULTS = None


def _build(repeats=1, hw_loop=1):
    # repeats > 1 emits the whole body multiple times; hw_loop > 1 wraps
    # the body in an on-device For_i loop. Both are used only by the
    # benchmarking harness to isolate device time from dispatch
    # overhead. Grading path always uses repeats=1, hw_loop=1.
    import concourse.bass as bass
    import concourse.bacc as bacc
    import concourse.mybir as mybir
    import concourse.tile as tile

    f32 = mybir.dt.float32
    f32r = mybir.dt.float32r

    nc = bacc.Bacc(
        "TRN2", target_bir_lowering=False, debug=False, num_devices=N_CORES
    )
    xp_d = nc.dram_tensor("xp", (NPC, C, HP, WP), f32r, kind="ExternalInput")
    wT_d = nc.dram_tensor("wT", (C, KH * KW, O), f32r, kind="ExternalInput")
    b_d = nc.dram_tensor("b2", (128, OC_TILES), f32, kind="ExternalInput")
    out_d = nc.dram_tensor("out", (NPC, O, H, W), f32, kind="ExternalOutput")

    with tile.TileContext(nc) as tc:
        with (
            tc.tile_pool(name="w", bufs=1) as wpool,
            tc.tile_pool(name="x", bufs=2) as xpool,
            tc.tile_pool(name="ps", bufs=4, space=bass.MemorySpace.PSUM) as pspool,
            tc.tile_pool(name="o", bufs=6) as opool,
        ):
            # Startup-critical DMA placement.  Tile deps are
            # sub-tile-range aware and there are two HWDGE rings (SP via
            # nc.sync, ACT via nc.scalar), so the first matmul's exact
            # dependencies -- input rows 0..9 (SP ring) and the oc=0
            # half of the weights (ACT ring) -- transfer in parallel and
            # land ~3us in.  Everything else queues behind them: stores
            # own the SP ring, later input loads the ACT ring.
            w_t = wpool.tile([C, KH * KW, O], f32r)
            b_t = wpool.tile([128, OC_TILES], f32)
            # first matmuls gate on taps 0-3 of oc=0 only (256KB), the
            # remaining taps stream in behind them.
            nc.scalar.dma_start(w_t[:, 0:4, 0:128], wT_d[:, 0:4, 0:128])
            nc.scalar.dma_start(w_t[:, 4:9, 0:128], wT_d[:, 4:9, 0:128])

            def body(first=False):
                for idx, n in enumerate(
                    [i % NPC for i in range(repeats * NPC)]
                ):
                    x_t = xpool.tile([C, HP, WP], f32r)
                    head = RPC + 2 * PAD  # rows needed by chunk 0
                    if first and idx == 0:
                        # image 0: rows split head/mid (SP ring) and
                        # tail (ACT ring); weights and bias on the ACT
                        # ring, all in deadline order.
                        mid = 34
                        nc.sync.dma_start(
                            x_t[:, 0:head, :], xp_d[n, :, 0:head, :]
                        )
                        nc.scalar.dma_start(
                            w_t[:, :, 128:256], wT_d[:, :, 128:256]
                        )
                        nc.sync.dma_start(
                            x_t[:, head:mid, :], xp_d[n, :, head:mid, :]
                        )
                        nc.scalar.dma_start(b_t[:], b_d[:])
                        nc.scalar.dma_start(
                            x_t[:, mid:HP, :], xp_d[n, :, mid:HP, :]
                        )
                    else:
                        nc.scalar.dma_start(x_t[:], xp_d[n])
                    for ch in range(N_CHUNKS):
                        y0 = ch * RPC
                        for oc in range(OC_TILES):
                            ps = pspool.tile([128, RPC, W], f32)
                            k = 0
                            for kh in range(KH):
                                for kw in range(KW):
                                    nc.tensor.matmul(
                                        ps[:],
                                        w_t[
                                            :, kh * KW + kw, oc * 128 : (oc + 1) * 128
                                        ],
                                        x_t[:, y0 + kh : y0 + kh + RPC, kw : kw + W],
                                        start=(k == 0),
                                        stop=(k == KH * KW - 1),
                                    )
                                    k += 1
                            o_t = opool.tile([128, RPC, W], f32)
                            bias_ap = b_t[:, oc : oc + 1]
                            out_ap = out_d[
                                n, oc * 128 : (oc + 1) * 128, y0 : y0 + RPC, :
                            ]
                            is_last = (
                                idx == repeats * NPC - 1
                                and ch == N_CHUNKS - 1
                                and oc == OC_TILES - 1
                            )
                            if is_last:
                                # final group: halve the eviction across
                                # ACT+DVE and the store across both
                                # HWDGE rings to shorten the kernel tail.
                                h = RPC // 2
                                nc.scalar.add(
                                    o_t[:, 0:h, :], ps[:, 0:h, :], bias_ap
                                )
                                nc.vector.tensor_scalar_add(
                                    o_t[:, h:RPC, :], ps[:, h:RPC, :], bias_ap
                                )
                                nc.sync.dma_start(
                                    out_ap[:, 0:h, :], o_t[:, 0:h, :]
                                )
                                nc.scalar.dma_start(
                                    out_ap[:, h:RPC, :], o_t[:, h:RPC, :]
                                )
                            elif (ch * OC_TILES + oc) % 2 == 0:
                                nc.scalar.add(o_t[:], ps[:], bias_ap)
                                nc.sync.dma_start(out_ap, o_t[:])
                            else:
                                nc.vector.tensor_scalar_add(
                                    o_t[:], ps[:], bias_ap
                                )
                                nc.sync.dma_start(out_ap, o_t[:])

            if hw_loop > 1:
                nc.scalar.dma_start(w_t[:, :, 128:256], wT_d[:, :, 128:256])
                nc.scalar.dma_start(b_t[:], b_d[:])
                with tc.For_i(0, hw_loop, 1):
                    body()
            else:
                body(first=True)
    nc.compile()
    return nc


def kernel(x, weight, bias):
    global LAST_RESULTS
    from concourse.bass_utils import run_bass_kernel_spmd

    x = np.asarray(x, dtype=np.float32)
    weight = np.asarray(weight, dtype=np.float32)
    bias = np.asarray(bias, dtype=np.float32)

    xp = np.zeros((N, C, HP, WP), np.float32)
    xp[:, :, PAD : PAD + H, PAD : PAD + W] = x
    # wT[i, kh*KW+kw, o] = weight[o, i, kh, kw]
    wT = np.ascontiguousarray(weight.transpose(1, 2, 3, 0)).reshape(C, KH * KW, O)
    # b2[p, oc] = bias[oc*128 + p]
    b2 = np.ascontiguousarray(bias.reshape(OC_TILES, 128).T)

    if "nc" not in _CACHE:
        _CACHE["nc"] = _build()
    nc = _CACHE["nc"]

    in_maps = [
        {"xp": xp[i * NPC : (i + 1) * NPC], "wT": wT, "b2": b2}
        for i in range(N_CORES)
    ]
    res = run_bass_kernel_spmd(nc, in_maps, core_ids=list(range(N_CORES)))
    LAST_RESULTS = res
    return np.concatenate([r["out"] for r in res.results], axis=0)



# revision 29
# speedup vs baseline: 1.0236x; 1.0236x over previous
"""Conv2d 3x3 (stride 1, pad 1) NCHW kernel for 8 Trainium2 NeuronCores.

Problem: x (32,128,56,56) f32, weight (256,128,3,3), bias (256,)
         -> out (32,256,56,56), same-padding conv + bias.

Strategy:
  - Data parallel: 4 images per core across 8 cores (batch shard).
  - Host pre-pads x to 58x58 and transposes weight to [Cin, kh*kw, Cout]
    so every DMA is large and contiguous.
  - Implicit GEMM: input channels (128) live on the SBUF partition dim.
    For each (kh, kw) of the 3x3 window the conv is a [128x128] weight
    matmul against a shifted spatial window of the padded image; the 9
    taps accumulate in PSUM.  Output tiling: 128 output channels x
    (8 rows x 56 cols) = free dim 448 per matmul (<=512 fp32 PSUM bank).
  - float32r matmuls: full PE rate at free dim >= 256 (1 cycle/row, vs 4
    for fp32), ~1.5e-4 rel err; accumulation is fp32 in PSUM.
  - Bias is fused into the PSUM->SBUF eviction (ACT/DVE alternating).
  - Startup: Tile deps are sub-tile-range aware, so the first matmul
    gates only on taps 0-3 of the oc=0 weights (ACT HWDGE ring) and
    input rows 0-9 of image 0 (SP ring), transferring in parallel.
    Stores own the SP ring, input loads the ACT ring.  The final
    group's eviction/store is split across ACT+DVE and both rings to
    shorten the kernel tail.

Measured (repeat-slope method, see bench.py): ~93 us/body steady state,
at the f32r PE roofline (504 matmuls x 448 cols / 2.4 GHz = 94.1 us);
cost-model single-exec estimate ~107 us including startup + drain tail.
"""

import os
import numpy as np

N_CORES = 8
N, C, H, W = 32, 128, 56, 56
O = 256
KH = KW = 3
PAD = 1
HP, WP = H + 2 * PAD, W + 2 * PAD  # 58, 58
NPC = N // N_CORES  # images per core = 4
RPC = 8  # output rows per chunk
N_CHUNKS = H // RPC  # 7
OC_TILES = O // 128  # 2

_CACHE = {}
LAST_RESULTS = None


def _build(repeats=1, hw_loop=1):
    # repeats > 1 emits the whole body multiple times; hw_loop > 1 wraps
    # the body in an on-device For_i loop. Both are used only by the
    # benchmarking harness to isolate device time from dispatch
    # overhead. Grading path always uses repeats=1, hw_loop=1.
    import concourse.bass as bass
    import concourse.bacc as bacc
    import concourse.mybir as mybir
    import concourse.tile as tile

    f32 = mybir.dt.float32
    f32r = mybir.dt.float32r

    nc = bacc.Bacc(
        "TRN2", target_bir_lowering=False, debug=False, num_devices=N_CORES
    )
    xp_d = nc.dram_tensor("xp", (NPC, C, HP, WP), f32r, kind="ExternalInput")
    wT_d = nc.dram_tensor("wT", (C, KH * KW, O), f32r, kind="ExternalInput")
    b_d = nc.dram_tensor("b2", (128, OC_TILES), f32, kind="ExternalInput")
    out_d = nc.dram_tensor("out", (NPC, O, H, W), f32, kind="ExternalOutput")

    with tile.TileContext(nc) as tc:
        with (
            tc.tile_pool(name="w", bufs=1) as wpool,
            tc.tile_pool(name="x", bufs=2) as xpool,
            tc.tile_pool(name="ps", bufs=4, space=bass.MemorySpace.PSUM) as pspool,
            tc.tile_pool(name="o", bufs=6) as opool,
        ):
            # Startup-critical DMA placement.  Tile deps are
            # sub-tile-range aware and there are two HWDGE rings (SP via
            # nc.sync, ACT via nc.scalar), so the first matmul's exact
            # dependencies -- input rows 0..9 (SP ring) and the oc=0
            # half of the weights (ACT ring) -- transfer in parallel and
            # land ~3us in.  Everything else queues behind them: stores
            # own the SP ring, later input loads the ACT ring.
            w_t = wpool.tile([C, KH * KW, O], f32r)
            b_t = wpool.tile([128, OC_TILES], f32)
            # PE p-state anchor: the tensor engine reaches full clock
            # 3us after its FIRST instruction (pe_busy_start is sticky
            # across idle gaps), so issue one tiny dummy matmul as early
            # as possible; the real matmuls then hit 2.4 GHz by ~4.2us.
            dummy = wpool.tile([128, 256], f32)
            psw = pspool.tile([128, 256], f32, name="psw", tag="psw", bufs=1)
            nc.vector.memset(dummy[:], 0.0)
            # 3 dummies end ~3.8us, just before the first real matmul:
            # the ramp anchor resets if the PE idles ~3us, so keep the
            # gap small while still spanning the ramp window.
            for _ in range(3):
                nc.tensor.matmul(
                    psw[:], dummy[:, 0:128], dummy[:], start=True, stop=True
                )
            # All DMA transfers serialize on one device in trigger
            # order, so issue in deadline order: kh=0 taps of oc=0 gate
            # the first matmuls, then the rest of oc=0, then oc=1 (needed
            # ~1.7us after the first matmul), then bias.
            nc.scalar.dma_start(w_t[:, 0:3, 0:128], wT_d[:, 0:3, 0:128])
            nc.scalar.dma_start(w_t[:, 3:9, 0:128], wT_d[:, 3:9, 0:128])

            def body(first=False):
                for idx, n in enumerate(
                    [i % NPC for i in range(repeats * NPC)]
                ):
                    x_t = xpool.tile([C, HP, WP], f32r)
                    head = RPC + 2 * PAD  # rows needed by chunk 0
                    if first and idx == 0:
                        # image 0: rows 0:10 ride the otherwise-empty SP
                        # ring (trigger lands before the weight
                        # triggers); later rows go on the ACT ring
                        # BEHIND the weights so they don't cut ahead of
                        # them in the serial DMA queue.
                        mid = 34
                        nc.sync.dma_start(
                            x_t[:, 0:head, :], xp_d[n, :, 0:head, :]
                        )
                        nc.scalar.dma_start(
                            w_t[:, :, 128:256], wT_d[:, :, 128:256]
                        )
                        nc.scalar.dma_start(b_t[:], b_d[:])
                        nc.scalar.dma_start(
                            x_t[:, head : head + 8, :],
                            xp_d[n, :, head : head + 8, :],
                        )
                        nc.scalar.dma_start(
                            x_t[:, head + 8 : mid, :],
                            xp_d[n, :, head + 8 : mid, :],
                        )
                        nc.scalar.dma_start(
                            x_t[:, mid:HP, :], xp_d[n, :, mid:HP, :]
                        )
                    else:
                        nc.scalar.dma_start(x_t[:], xp_d[n])
                    for ch in range(N_CHUNKS):
                        y0 = ch * RPC
                        for oc in range(OC_TILES):
                            is_last = (
                                idx == repeats * NPC - 1
                                and ch == N_CHUNKS - 1
                                and oc == OC_TILES - 1
                            )
                            ps = pspool.tile([128, RPC, W], f32)
                            k = 0
                            for kh in range(KH):
                                for kw in range(KW):
                                    nc.tensor.matmul(
                                        ps[:],
                                        w_t[
                                            :, kh * KW + kw, oc * 128 : (oc + 1) * 128
                                        ],
                                        x_t[:, y0 + kh : y0 + kh + RPC, kw : kw + W],
                                        start=(k == 0),
                                        stop=(k == KH * KW - 1),
                                    )
                                    k += 1
                            o_t = opool.tile([128, RPC, W], f32)
                            bias_ap = b_t[:, oc : oc + 1]
                            out_ap = out_d[
                                n, oc * 128 : (oc + 1) * 128, y0 : y0 + RPC, :
                            ]
                            if is_last:
                                # final group: halve the eviction across
                                # ACT+DVE and the store across both
                                # HWDGE rings to shorten the kernel tail.
                                h = RPC // 2
                                nc.scalar.add(
                                    o_t[:, 0:h, :], ps[:, 0:h, :], bias_ap
                                )
                                nc.vector.tensor_scalar_add(
                                    o_t[:, h:RPC, :], ps[:, h:RPC, :], bias_ap
                                )
                                nc.sync.dma_start(
                                    out_ap[:, 0:h, :], o_t[:, 0:h, :]
                                )
                                nc.scalar.dma_start(
                                    out_ap[:, h:RPC, :], o_t[:, h:RPC, :]
                                )
                            elif (ch * OC_TILES + oc) % 2 == 0:
                                nc.scalar.add(o_t[:], ps[:], bias_ap)
                                nc.sync.dma_start(out_ap, o_t[:])
                            else:
                                nc.vector.tensor_scalar_add(
                                    o_t[:], ps[:], bias_ap
                                )
                                nc.sync.dma_start(out_ap, o_t[:])

            if hw_loop > 1:
                nc.scalar.dma_start(w_t[:, :, 128:256], wT_d[:, :, 128:256])
                nc.scalar.dma_start(b_t[:], b_d[:])
                with tc.For_i(0, hw_loop, 1):
                    body()
            else:
                body(first=True)
    nc.compile()
    return nc


def kernel(x, weight, bias):
    global LAST_RESULTS
    from concourse.bass_utils import run_bass_kernel_spmd

    x = np.asarray(x, dtype=np.float32)
    weight = np.asarray(weight, dtype=np.float32)
    bias = np.asarray(bias, dtype=np.float32)

    xp = np.zeros((N, C, HP, WP), np.float32)
    xp[:, :, PAD : PAD + H, PAD : PAD + W] = x
    # wT[i, kh*KW+kw, o] = weight[o, i, kh, kw]
    wT = np.ascontiguousarray(weight.transpose(1, 2, 3, 0)).reshape(C, KH * KW, O)
    # b2[p, oc] = bias[oc*128 + p]
    b2 = np.ascontiguousarray(bias.reshape(OC_TILES, 128).T)

    if "nc" not in _CACHE:
        _CACHE["nc"] = _build()
    nc = _CACHE["nc"]

    in_maps = [
        {"xp": xp[i * NPC : (i + 1) * NPC], "wT": wT, "b2": b2}
        for i in range(N_CORES)
    ]
    res = run_bass_kernel_spmd(nc, in_maps, core_ids=list(range(N_CORES)))
    LAST_RESULTS = res
    return np.concatenate([r["out"] for r in res.results], axis=0)

